# revision 1
# baseline (speedup 1.0000x reference)
"""Trainium2 kernel for nn_Block1SyntaxEngine_85959475462663
(6-layer dense transformer, B=2 T=1024 D=1024 H=16 DFF=2048, fp32 ref).

Distribution: 2-way data-parallel over batch (core groups [0-3], [4-7]) x
4-way Megatron tensor-parallel inside each group (4 heads + 512 d_ff columns
per core). fp16 matmul inputs (fp32 PSUM accumulation), fp32 residual stream
resident in SBUF, LayerNorm scale/bias folded into the following weights on
the host, softmax normalization folded into the Wout matmul epilogue, two
fp16 AllReduces per layer. Causal attention is block-sparse, computed in
k-major orientation so probabilities feed attn@v as lhsT without transposes;
activation transposes use the fp16 DMA-transpose path via a DRAM bounce.

Self-contained: only needs numpy/jax/concourse (the trn_rl_repo toolchain
on sys.path) and 8 visible neuron cores.
"""
import contextlib
import time

import numpy as np

import concourse.bass as bass
import concourse.mybir as mybir
import concourse.tile as tile
from concourse import bacc

P = 128
B, T, D, H, L, V = 2, 1024, 1024, 16, 6, 32000
DH = D // H            # 64
DFF = 2 * D            # 2048
NCORES = 8
NG = 4                 # tensor-parallel degree (cores per group)
HR = H // NG           # heads per core
FR = DFF // NG         # ffn columns per core
TT = T // P            # token tiles
KT = D // P            # contraction tiles over D

f16 = mybir.dt.float16
f32 = mybir.dt.float32
AF = mybir.ActivationFunctionType
ALU = mybir.AluOpType
EPS = 1e-5
SIM_GELU_SUBST = False   # True: use Sigmoid instead of Gelu (sim lacks Gelu)
GROUPS = [[0, 1, 2, 3], [4, 5, 6, 7]]


def build_nc():
    nc = bacc.Bacc()
    dp = dict(
        emb=nc.declare_dram_parameter("emb", [V, D], f16, isOutput=False),
        ids=nc.declare_dram_parameter("ids", [T, 1], mybir.dt.int32, isOutput=False),
        pos=nc.declare_dram_parameter("pos", [T, D], f16, isOutput=False),
        wqk=nc.declare_dram_parameter("wqk", [L, D, 4, P], f16, isOutput=False),
        wv=nc.declare_dram_parameter("wv", [L, D, HR * DH], f16, isOutput=False),
        wout=nc.declare_dram_parameter("wout", [L, HR * DH, D], f16, isOutput=False),
        w1=nc.declare_dram_parameter("w1", [L, D, FR], f16, isOutput=False),
        w2=nc.declare_dram_parameter("w2", [L, FR, D], f16, isOutput=False),
        bqk=nc.declare_dram_parameter("bqk", [L, 4, P], f32, isOutput=False),
        bv=nc.declare_dram_parameter("bv", [L, HR * DH], f16, isOutput=False),
        b1=nc.declare_dram_parameter("b1", [L, 4, P], f32, isOutput=False),
        lnf_sb=nc.declare_dram_parameter("lnf_sb", [2, D], f32, isOutput=False),
        mask=nc.declare_dram_parameter("mask", [P, P], f16, isOutput=False),
        out=nc.declare_dram_parameter("out", [T, D], f32, isOutput=True),
    )
    with tile.TileContext(nc) as tc:
        _body(nc, tc, dp)
    nc.finalize()
    return nc


def _ln_stats(nc, sp, sp_big_sqf, eps_t, src_ap):
    """Per-token -mean and 1/std ([P,1] f32 tiles) of a [P, D] fp32 tile."""
    s1 = sp.tile([P, 1], f32, tag="ln_s1")
    nc.vector.reduce_sum(s1[:], src_ap, axis=mybir.AxisListType.X)
    nm = sp.tile([P, 1], f32, tag="ln_nm")
    nc.scalar.mul(nm[:], s1[:], -1.0 / D)
    sq = sp.tile([P, 1], f32, tag="ln_sq")
    sqf = sp_big_sqf.tile([P, D], f32, tag="ln_sqf")
    nc.scalar.activation(sqf[:], src_ap, AF.Square, accum_out=sq[:])
    var = sp.tile([P, 1], f32, tag="ln_var")
    nc.vector.tensor_mul(var[:], nm[:], nm[:])
    tmp = sp.tile([P, 1], f32, tag="ln_tmp")
    nc.vector.tensor_scalar_mul(tmp[:], sq[:], 1.0 / D)
    nc.vector.tensor_sub(var[:], tmp[:], var[:])
    std = sp.tile([P, 1], f32, tag="ln_std")
    nc.scalar.activation(std[:], var[:], AF.Sqrt, bias=eps_t[:])
    rstd = sp.tile([P, 1], f32, tag="ln_rstd")
    nc.vector.reciprocal(rstd[:], std[:])
    return nm, rstd


def _body(nc, tc, dp):
    ctx = contextlib.ExitStack()
    with ctx:
        xp = ctx.enter_context(tc.tile_pool(name="xp", bufs=1))
        cst = ctx.enter_context(tc.tile_pool(name="cst", bufs=1))
        wp = ctx.enter_context(tc.tile_pool(name="wp", bufs=1))
        hp = ctx.enter_context(tc.tile_pool(name="hp", bufs=2))
        ep = ctx.enter_context(tc.tile_pool(name="ep", bufs=1))
        ap_ = ctx.enter_context(tc.tile_pool(name="ap", bufs=1))
        pp = ctx.enter_context(tc.tile_pool(name="pp", bufs=4))
        sp = ctx.enter_context(tc.tile_pool(name="sp", bufs=3))
        bigt = ctx.enter_context(tc.tile_pool(name="bigt", bufs=1))
        dmp = ctx.enter_context(tc.tile_pool(name="dmp", bufs=3, space="DRAM"))
        ps512 = ctx.enter_context(tc.tile_pool(name="ps512", bufs=3, space="PSUM"))
        ps256 = ctx.enter_context(tc.tile_pool(name="ps256", bufs=2, space="PSUM"))
        psav = ctx.enter_context(tc.tile_pool(name="psav", bufs=2, space="PSUM"))

        # ---- constants ----
        mask16 = cst.tile([P, P], f16)
        nc.sync.dma_start(mask16[:], dp["mask"][:])
        lnf_t = cst.tile([P, 2, D], f32)
        nc.sync.dma_start(lnf_t[:, 0, :], dp["lnf_sb"][0, None, :].to_broadcast((P, D)))
        nc.sync.dma_start(lnf_t[:, 1, :], dp["lnf_sb"][1, None, :].to_broadcast((P, D)))
        eps_t = cst.tile([P, 1], f32)
        nc.vector.memset(eps_t[:], EPS)

        x = xp.tile([P, TT, D], f32)   # fp32 residual, persistent

        # ---- embeddings ----
        for tt in range(TT):
            gt = bigt.tile([P, D], f16, tag="gather")
            idt = sp.tile([P, 1], mybir.dt.int32, tag="ids")
            nc.sync.dma_start(idt[:], dp["ids"][tt * P:(tt + 1) * P, :])
            nc.gpsimd.indirect_dma_start(
                out=gt[:], out_offset=None, in_=dp["emb"][:, :],
                in_offset=bass.IndirectOffsetOnAxis(ap=idt[:, :1], axis=0),
            )
            pt = bigt.tile([P, D], f16, tag="pos")
            nc.sync.dma_start(pt[:], dp["pos"][tt * P:(tt + 1) * P, :])
            nc.vector.tensor_copy(x[:, tt, :], pt[:])
            nc.vector.tensor_add(x[:, tt, :], x[:, tt, :], gt[:])

        # v_ext: [P, TT(j), HR, DH+1] fp16; last col stays 1.0
        v_ext = ep.tile([P, TT, HR, DH + 1], f16, tag="vext")
        nc.vector.memset(v_ext[:], 0.0)
        nc.vector.memset(v_ext[:, :, :, DH], 1.0)

        HT = TT // 2          # token tiles per half
        HTOK = HT * P         # tokens per half

        def layernorm_T_half(hxT, half):
            """LN(x[half]) -> fp16 -> transposed into hxT[:, :, half cols]."""
            xh_dram = dmp.tile([HTOK, D], f16, tag="xh_dram")
            xh = bigt.tile([P, HT, D], f16, tag="ln_xh")
            for i in range(HT):
                tt = half * HT + i
                nm, rstd = _ln_stats(nc, sp, bigt, eps_t, x[:, tt, :])
                nc.vector.tensor_scalar(
                    xh[:, i, :], x[:, tt, :], nm[:], rstd[:], ALU.add, ALU.mult)
            nc.sync.dma_start(xh_dram[:].rearrange("(o p) d -> p o d", p=P), xh[:])
            for k in range(KT):
                nc.sync.dma_start_transpose(
                    hxT[:, k, half * HTOK:(half + 1) * HTOK],
                    xh_dram[:, k * P:(k + 1) * P])

        def allreduce_half(part_tile, half):
            """AR the half-partial and add into x[half]. part_tile: [P, HT, D]."""
            ar_i = dmp.tile([HTOK, D], f16, tag="ar_in")
            ar_o = dmp.tile([HTOK, D], f16, tag="ar_out")
            nc.sync.dma_start(ar_i[:].rearrange("(o p) d -> p o d", p=P), part_tile[:])
            nc.gpsimd.collective_compute(
                "AllReduce", ALU.add, replica_groups=GROUPS,
                ins=[ar_i[:]], outs=[ar_o[:]],
            )
            d16 = pp.tile([P, HT, D], f16, tag="part")
            nc.sync.dma_start(d16[:], ar_o[:].rearrange("(o p) d -> p o d", p=P))
            for i in range(HT):
                tt = half * HT + i
                nc.vector.tensor_add(x[:, tt, :], x[:, tt, :], d16[:, i, :])

        # ================= layers =================
        for l in range(L):
            hxT = hp.tile([P, KT, T], f16, tag="hxT")
            layernorm_T_half(hxT, 0)
            layernorm_T_half(hxT, 1)

            wqk_t = wp.tile([P, KT, 4, P], f16, tag="wqk")
            nc.sync.dma_start(wqk_t[:], dp["wqk"][l].rearrange("(kt p) m n -> p kt m n", p=P))
            wv_t = wp.tile([P, KT, HR * DH], f16, tag="wv")
            nc.sync.dma_start(wv_t[:], dp["wv"][l].rearrange("(kt p) n -> p kt n", p=P))
            wout_t = wp.tile([P, 2, D], f16, tag="wout")
            nc.sync.dma_start(wout_t[:], dp["wout"][l].rearrange("(kt p) n -> p kt n", p=P))
            w1_t = wp.tile([P, KT, FR], f16, tag="w1")
            nc.sync.dma_start(w1_t[:], dp["w1"][l].rearrange("(kt p) n -> p kt n", p=P))
            w2_t = wp.tile([P, 4, D], f16, tag="w2")
            nc.sync.dma_start(w2_t[:], dp["w2"][l].rearrange("(kt p) n -> p kt n", p=P))
            bqk_t = wp.tile([P, 4], f32, tag="bqk")
            nc.sync.dma_start(bqk_t[:], dp["bqk"][l].rearrange("m p -> p m"))
            bv_t = wp.tile([P, HR * DH], f16, tag="bv")
            nc.sync.dma_start(bv_t[:], dp["bv"][l, None, :].to_broadcast((P, HR * DH)))
            b1_t = wp.tile([P, 4], f32, tag="b1")
            nc.sync.dma_start(b1_t[:], dp["b1"][l].rearrange("m p -> p m"))

            # q/k projections, feature-major [P, 4, T]
            qkT = ap_.tile([P, 4, T], f16, tag="qkT")
            for mt in range(4):
                for c in range(2):
                    pt_ = ps512.tile([P, 512], f32, tag="mm512")
                    for k in range(KT):
                        nc.tensor.matmul(
                            pt_[:], wqk_t[:, k, mt, :],
                            hxT[:, k, c * 512:(c + 1) * 512],
                            start=(k == 0), stop=(k == KT - 1))
                    nc.vector.tensor_scalar_add(
                        qkT[:, mt, c * 512:(c + 1) * 512], pt_[:], bqk_t[:, mt, None])

            # v projection, token-major, into v_ext
            for tt in range(TT):
                pv = ps256.tile([P, HR * DH], f32, tag="mm256")
                for k in range(KT):
                    nc.tensor.matmul(
                        pv[:], hxT[:, k, tt * P:(tt + 1) * P], wv_t[:, k, :],
                        start=(k == 0), stop=(k == KT - 1))
                vb = sp.tile([P, HR * DH], f16, tag="vtmp")
                nc.vector.tensor_add(vb[:], pv[:], bv_t[:])
                nc.vector.tensor_copy(
                    v_ext[:, tt, :, 0:DH],
                    vb[:].rearrange("p (h d) -> p h d", h=HR))

            # attention
            o16 = ap_.tile([P, TT, HR * DH], f16, tag="o16")
            for h in range(HR):
                mt_q = 2 * (h // 2)
                lo = DH * (h % 2)
                E16 = ep.tile([P, TT, T], f16, tag="E16")
                for c in range(2):
                    for j in range(4 * (c + 1)):
                        pe = ps512.tile([P, 512], f32, tag="mm512")
                        nc.tensor.matmul(
                            pe[:],
                            qkT[lo:lo + DH, mt_q + 1, j * P:(j + 1) * P],
                            qkT[lo:lo + DH, mt_q, c * 512:(c + 1) * 512],
                            start=True, stop=True)
                        nc.scalar.activation(
                            E16[:, j, c * 512:(c + 1) * 512], pe[:],
                            AF.Exp, scale=float(1.0 / np.sqrt(DH)))
                for t in range(TT):
                    nc.vector.tensor_mul(
                        E16[:, t, t * P:(t + 1) * P],
                        E16[:, t, t * P:(t + 1) * P], mask16[:])
                for j in range(1, 4):
                    for qt in range(0, j):
                        nc.vector.memset(E16[:, j, qt * P:(qt + 1) * P], 0.0)
                for j in range(5, TT):
                    for qt in range(4, j):
                        nc.vector.memset(E16[:, j, qt * P:(qt + 1) * P], 0.0)
                for qt in range(TT):
                    po = psav.tile([P, DH + 1], f32, tag="mmav")
                    for j in range(qt + 1):
                        nc.tensor.matmul(
                            po[:], E16[:, j, qt * P:(qt + 1) * P],
                            v_ext[:, j, h, :],
                            start=(j == 0), stop=(j == qt))
                    rn = sp.tile([P, 1], f32, tag="rn")
                    nc.vector.reciprocal(rn[:], po[:, DH:DH + 1])
                    nc.vector.tensor_scalar_mul(
                        o16[:, qt, h * DH:(h + 1) * DH], po[:, 0:DH], rn[:])

            # o -> oT via DRAM roundtrip transpose; Wout + AR per half
            oT = ap_.tile([P, 2, T], f16, tag="oT")
            for half in range(2):
                o_dram = dmp.tile([HTOK, HR * DH], f16, tag="o_dram")
                nc.sync.dma_start(
                    o_dram[:].rearrange("(o p) d -> p o d", p=P),
                    o16[:, half * HT:(half + 1) * HT, :])
                for k in range(2):
                    nc.sync.dma_start_transpose(
                        oT[:, k, half * HTOK:(half + 1) * HTOK],
                        o_dram[:, k * P:(k + 1) * P])
            hx2T = hp.tile([P, KT, T], f16, tag="hxT")
            for half in range(2):
                part = pp.tile([P, HT, D], f16, tag="part")
                for i in range(HT):
                    tt = half * HT + i
                    for c in range(2):
                        pw = ps512.tile([P, 512], f32, tag="mm512")
                        for k in range(2):
                            nc.tensor.matmul(
                                pw[:], oT[:, k, tt * P:(tt + 1) * P],
                                wout_t[:, k, c * 512:(c + 1) * 512],
                                start=(k == 0), stop=(k == 1))
                        nc.vector.tensor_copy(part[:, i, c * 512:(c + 1) * 512], pw[:])
                allreduce_half(part, half)
                layernorm_T_half(hx2T, half)

            # FFN per half: W1+gelu then W2 partial + AR
            h1gT = ap_.tile([P, 4, T], f16, tag="h1gT")
            for half in range(2):
                for mt in range(4):
                    pf = ps512.tile([P, 512], f32, tag="mm512")
                    for k in range(KT):
                        nc.tensor.matmul(
                            pf[:], w1_t[:, k, mt * P:(mt + 1) * P],
                            hx2T[:, k, half * 512:(half + 1) * 512],
                            start=(k == 0), stop=(k == KT - 1))
                    nc.scalar.activation(
                        h1gT[:, mt, half * 512:(half + 1) * 512], pf[:],
                        AF.Sigmoid if SIM_GELU_SUBST else AF.Gelu,
                        bias=b1_t[:, mt, None])
                part2 = pp.tile([P, HT, D], f16, tag="part")
                for i in range(HT):
                    tt = half * HT + i
                    for c in range(2):
                        pw = ps512.tile([P, 512], f32, tag="mm512")
                        for k in range(4):
                            nc.tensor.matmul(
                                pw[:], h1gT[:, k, tt * P:(tt + 1) * P],
                                w2_t[:, k, c * 512:(c + 1) * 512],
                                start=(k == 0), stop=(k == 3))
                        nc.vector.tensor_copy(part2[:, i, c * 512:(c + 1) * 512], pw[:])
                allreduce_half(part2, half)

        # ---- final layernorm + output ----
        for tt in range(TT):
            nm, rstd = _ln_stats(nc, sp, bigt, eps_t, x[:, tt, :])
            xh = bigt.tile([P, D], f32, tag="ln_xhf")
            nc.vector.tensor_scalar(
                xh[:], x[:, tt, :], nm[:], rstd[:], ALU.add, ALU.mult)
            zo = bigt.tile([P, D], f32, tag="zo")
            nc.vector.tensor_mul(zo[:], xh[:], lnf_t[:, 0, :])
            nc.vector.tensor_add(zo[:], zo[:], lnf_t[:, 1, :])
            nc.sync.dma_start(dp["out"][tt * P:(tt + 1) * P, :], zo[:])


# ======================= host side =======================

def _prep_inputs(input_ids, token_emb, pos_emb, ln1_s, ln1_b, Wqkv, Wout,
                 ln2_s, ln2_b, W1, W2, lnf_s, lnf_b):
    emb16 = np.asarray(token_emb, np.float16)
    pos16 = np.asarray(pos_emb, np.float16)
    ids_np = np.asarray(input_ids).astype(np.int32)
    # E^T[j, q] is valid where j <= q: upper triangle in (j=partition, q=free)
    mask_np = np.triu(np.ones((P, P), np.float32)).astype(np.float16)
    Wqkv64 = np.asarray(Wqkv, np.float64)
    W164 = np.asarray(W1, np.float64)
    Wqkv_f = Wqkv64 * np.asarray(ln1_s, np.float64)[:, :, None]
    bqkv_f = np.einsum("ld,ldn->ln", np.asarray(ln1_b, np.float64), Wqkv64)
    W1_f = W164 * np.asarray(ln2_s, np.float64)[:, :, None]
    b1_f = np.einsum("ld,ldn->ln", np.asarray(ln2_b, np.float64), W164)
    lnf_sb = np.stack([np.asarray(lnf_s, np.float32),
                       np.asarray(lnf_b, np.float32)])

    in_maps = []
    for core in range(NCORES):
        g, r = divmod(core, NG)
        heads = [HR * r + i for i in range(HR)]
        wqk_np = np.empty((L, D, 4, P), np.float16)
        bqk_np = np.empty((L, 4, P), np.float32)
        for ht in range(2):
            h0, h1 = heads[2 * ht], heads[2 * ht + 1]
            qcols = np.r_[DH * h0:DH * h0 + DH, DH * h1:DH * h1 + DH]
            kcols = D + qcols
            wqk_np[:, :, 2 * ht, :] = Wqkv_f[:, :, qcols].astype(np.float16)
            wqk_np[:, :, 2 * ht + 1, :] = Wqkv_f[:, :, kcols].astype(np.float16)
            bqk_np[:, 2 * ht, :] = bqkv_f[:, qcols].astype(np.float32)
            bqk_np[:, 2 * ht + 1, :] = bqkv_f[:, kcols].astype(np.float32)
        vcols = np.r_[tuple(np.arange(2 * D + DH * h, 2 * D + DH * h + DH)
                            for h in heads)]
        orows = np.r_[tuple(np.arange(DH * h, DH * h + DH) for h in heads)]
        in_maps.append(dict(
            emb=emb16, ids=ids_np[g][:, None], pos=pos16,
            wqk=wqk_np,
            wv=Wqkv_f[:, :, vcols].astype(np.float16),
            wout=np.asarray(Wout, np.float16)[:, orows, :],
            w1=W1_f[:, :, FR * r:FR * (r + 1)].astype(np.float16),
            w2=np.asarray(W2, np.float16)[:, FR * r:FR * (r + 1), :],
            bqk=bqk_np,
            bv=bqkv_f[:, vcols].astype(np.float16),
            b1=b1_f[:, FR * r:FR * (r + 1)].astype(np.float32).reshape(L, 4, P),
            lnf_sb=lnf_sb, mask=mask_np,
        ))
    return in_maps


# ---------- compile-once / run-many PJRT runner (vendored) ----------

class SpmdRunner:
    def __init__(self, nc, n_cores=8):
        import jax
        from jax.sharding import Mesh, PartitionSpec
        from jax.experimental.shard_map import shard_map
        from concourse.bass2jax import (
            _bass_exec_p, install_neuronx_cc_hook, partition_id_tensor)
        self.jax = jax
        self.PartitionSpec = PartitionSpec
        install_neuronx_cc_hook()
        if not nc.is_finalized():
            nc.finalize()
        self.n_cores = n_cores
        partition_name = (
            nc.partition_id_tensor.name if nc.partition_id_tensor else None)
        in_names, out_names, out_avals, zero_outs = [], [], [], []
        for alloc in nc.m.functions[0].allocations:
            if not isinstance(alloc, mybir.MemoryLocationSet):
                continue
            name = alloc.memorylocations[0].name
            if alloc.kind == "ExternalInput":
                if name != partition_name:
                    in_names.append(name)
            elif alloc.kind == "ExternalOutput":
                out_names.append(name)
                shape = tuple(alloc.tensor_shape)
                dtype = mybir.dt.np(alloc.dtype)
                out_avals.append(jax.core.ShapedArray(shape, dtype))
                zero_outs.append(np.zeros(shape, dtype))
        self.in_names, self.out_names = in_names, out_names
        self.out_avals, self.zero_outs = out_avals, zero_outs
        n_params, n_outs = len(in_names), len(out_avals)
        self.n_params = n_params
        all_in = in_names + out_names + (
            [partition_name] if partition_name else [])
        donate = tuple(range(n_params, n_params + n_outs))

        def _b(*args):
            ops = list(args)
            if partition_name:
                ops.append(partition_id_tensor())
            return tuple(_bass_exec_p.bind(
                *ops, out_avals=tuple(out_avals), in_names=tuple(all_in),
                out_names=tuple(out_names), lowering_input_output_aliases=(),
                sim_require_finite=True, sim_require_nnan=True, nc=nc))

        devices = jax.devices()[:n_cores]
        self.mesh = Mesh(np.asarray(devices), ("core",))
        specs = (PartitionSpec("core"),)
        self.sharded = jax.jit(
            shard_map(_b, mesh=self.mesh,
                      in_specs=specs * (n_params + n_outs),
                      out_specs=specs * len(out_names), check_rep=False),
            donate_argnums=donate, keep_unused=True)
        self._dev_inputs = None

    def _zeros(self):
        return [np.zeros((self.n_cores * z.shape[0], *z.shape[1:]), z.dtype)
                for z in self.zero_outs]

    def stage_inputs(self, in_maps):
        jax, PS = self.jax, self.PartitionSpec
        per_core = [[np.asarray(m[n]) for n in self.in_names] for m in in_maps]
        concat = [np.concatenate([per_core[c][i] for c in range(self.n_cores)],
                                 axis=0) for i in range(self.n_params)]
        sh = jax.sharding.NamedSharding(self.mesh, PS("core"))
        self._dev_inputs = [jax.device_put(a, sh) for a in concat]
        for a in self._dev_inputs:
            a.block_until_ready()

    def run(self, in_maps=None):
        if in_maps is not None:
            self.stage_inputs(in_maps)
        outs = self.sharded(*self._dev_inputs, *self._zeros())
        out_np = [np.asarray(a) for a in outs]
        return [{n: out_np[i].reshape(self.n_cores, *self.out_avals[i].shape)[c]
                 for i, n in enumerate(self.out_names)}
                for c in range(self.n_cores)]

    def time_exec(self, iters=8, warmup=2):
        jax, PS = self.jax, self.PartitionSpec
        sh = jax.sharding.NamedSharding(self.mesh, PS("core"))
        zsets = [[jax.device_put(z, sh) for z in self._zeros()]
                 for _ in range(warmup + iters)]
        for zs in zsets:
            for z in zs:
                z.block_until_ready()
        outs = []
        for i in range(warmup):
            outs.append(self.sharded(*self._dev_inputs, *zsets[i]))
        for o in outs[-1]:
            o.block_until_ready()
        t0 = time.perf_counter()
        outs = []
        for i in range(iters):
            outs.append(self.sharded(*self._dev_inputs, *zsets[warmup + i]))
        for o in outs[-1]:
            o.block_until_ready()
        return (time.perf_counter() - t0) / iters


_RUNNER = None


def get_runner():
    global _RUNNER
    if _RUNNER is None:
        _RUNNER = SpmdRunner(build_nc(), NCORES)
    return _RUNNER


def kernel(**inputs) -> np.ndarray:
    in_maps = _prep_inputs(**{k: np.asarray(v) for k, v in inputs.items()})
    res = get_runner().run(in_maps)
    out = np.empty((B, T, D), np.float32)
    out[0] = res[0]["out"]
    out[1] = res[NG]["out"]
    return out



# revision 6
# speedup vs baseline: 1.5176x; 1.5176x over previous
"""Trainium2 kernel v2 for nn_Block1SyntaxEngine_85959475462663
(6-layer dense transformer, B=2 T=1024 D=1024 H=16 DFF=2048, fp32 ref).

Distribution: sequence-parallel. 8 cores = 2 batch groups x 4 sequence
shards. Core (g, r) owns the 256 tokens `r::4` of batch element g
(interleaved assignment -> every core has the same causal block
structure, so one uniform program runs on all cores; the causal
diagonal masks are per-core DATA). Each core computes ALL heads and the
FULL FFN for its tokens with replicated weights. The only collective is
one K/V AllGather per layer within each 4-core group. Embedding gather
happens on the host; LN1/LN2 scale+bias are folded into the following
projection weights on the host.

Per layer:
  LN1 (bn_stats) -> h fp16 -> DRAM bounce -> fp16 DMA-transpose -> hT
  k,q feature-major (weights stationary), v token-major (+ones column)
  k/v roundtrip through one AllGather -> kT_all / v_ext (all 1024 keys)
  scores E^T = kT_h^T q, k-major, head pairs packed in partition halves
  exp on the scalar engine covering a head pair per instruction
  attn@v with E^T stationary -> o token-major; 1/rowsum per q from the
  ones column; o -> DRAM bounce -> oT -> Wout; residual adds in fp32
  LN2 -> h2T -> W1 feature-major -> gelu(+folded bias) -> W2 -> residual
"""
import contextlib
import time

import numpy as np

import concourse.bass as bass
import concourse.mybir as mybir
import concourse.tile as tile
from concourse import bacc

P = 128
B, T, D, H, L, V = 2, 1024, 1024, 16, 6, 32000
DH = D // H            # 64
DFF = 2 * D            # 2048
NCORES = 8
NG = 4                 # cores per sequence group (one batch element)
NTOK = 256             # tokens owned per core
KT = D // P            # 8 contraction tiles over D
FC1 = DFF // P         # 16 ff chunks
VROW = H * (DH + 1)    # 1040: v_ext row per token tile
KSZ = KT * P * NTOK    # fp16 elems of kT_own in the AG buffer
VSZ = 2 * P * VROW     # fp16 elems of v_ext_own in the AG buffer
AGN = KSZ + VSZ

f16 = mybir.dt.float16
f32 = mybir.dt.float32
f8 = mybir.dt.float8e4
AF = mybir.ActivationFunctionType
ALU = mybir.AluOpType
EPS = 1e-5
SIM_GELU_SUBST = False   # True: use Sigmoid instead of Gelu (sim lacks Gelu)
NO_COMM = False
GROUPS = [[0, 1, 2, 3], [4, 5, 6, 7]]


def build_nc():
    nc = bacc.Bacc()
    dp = dict(
        x0=nc.declare_dram_parameter("x0", [P, 2, D], f32, isOutput=False),
        wq=nc.declare_dram_parameter("wq", [L, P, KT, D], f16, isOutput=False),
        wk=nc.declare_dram_parameter("wk", [L, P, KT, D], f16, isOutput=False),
        wv=nc.declare_dram_parameter("wv", [L, P, KT, D], f16, isOutput=False),
        wo=nc.declare_dram_parameter("wo", [L, P, KT, D], f16, isOutput=False),
        w1=nc.declare_dram_parameter("w1", [L, P, KT, DFF], f16, isOutput=False),
        w2=nc.declare_dram_parameter("w2", [L, P, FC1, D], f16, isOutput=False),
        bq=nc.declare_dram_parameter("bq", [L, P, KT], f32, isOutput=False),
        bk=nc.declare_dram_parameter("bk", [L, P, KT], f32, isOutput=False),
        bv=nc.declare_dram_parameter("bv", [L, D], f16, isOutput=False),
        b1=nc.declare_dram_parameter("b1", [L, P, FC1], f32, isOutput=False),
        lnf=nc.declare_dram_parameter("lnf", [2, D], f16, isOutput=False),
        mask=nc.declare_dram_parameter("mask", [P, NG, P], f16, isOutput=False),
        out=nc.declare_dram_parameter("out", [NTOK, D], f16, isOutput=True),
    )
    with tile.TileContext(nc) as tc:
        _body(nc, tc, dp)
    nc.finalize()
    return nc


def _layernorm(nc, sp, eps_t, x, h16):
    """LN both token tiles of x [P,2,D] f32 -> h16 [P,2,D] f16 (no scale)."""
    mv = sp.tile([P, 2, 2], f32, tag="ln_mv")
    for tb in range(2):
        st = sp.tile([P, 2, 6], f32, tag="ln_st")
        for g in range(2):
            nc.vector.bn_stats(st[:, g, :], x[:, tb, g * 512:(g + 1) * 512])
        nc.vector.bn_aggr(mv[:, tb, :], st[:])
    std = sp.tile([P, 2], f32, tag="ln_std")
    nc.scalar.activation(std[:], mv[:, :, 1], AF.Sqrt, bias=eps_t[:])
    rstd = sp.tile([P, 2], f32, tag="ln_rstd")
    nc.vector.reciprocal(rstd[:], std[:])
    for tb in range(2):
        nc.vector.tensor_scalar(
            h16[:, tb, :], x[:, tb, :], mv[:, tb, 0:1], rstd[:, tb:tb + 1],
            ALU.subtract, ALU.mult)


def _body(nc, tc, dp):
    ctx = contextlib.ExitStack()
    with ctx:
        cst = ctx.enter_context(tc.tile_pool(name="cst", bufs=1))
        wp = ctx.enter_context(tc.tile_pool(name="wp", bufs=1))
        xp = ctx.enter_context(tc.tile_pool(name="xp", bufs=1))
        ap_ = ctx.enter_context(tc.tile_pool(name="ap", bufs=1))
        ep = ctx.enter_context(tc.tile_pool(name="ep", bufs=1))
        sp = ctx.enter_context(tc.tile_pool(name="sp", bufs=2))
        dmp = ctx.enter_context(tc.tile_pool(name="dmp", bufs=2, space="DRAM"))
        psSC = ctx.enter_context(tc.tile_pool(name="psSC", bufs=2, space="PSUM"))
        psA = ctx.enter_context(tc.tile_pool(name="psA", bufs=2, space="PSUM"))
        psAV = ctx.enter_context(tc.tile_pool(name="psAV", bufs=2, space="PSUM"))

        # ---- constants ----
        maskd = cst.tile([P, NG, P], f16)
        nc.sync.dma_start(maskd[:], dp["mask"][:])
        lnf_t = cst.tile([P, 2, D], f16)
        for i in range(2):
            nc.sync.dma_start(
                lnf_t[:, i, :], dp["lnf"][i, None, :].to_broadcast((P, D)))
        eps_t = cst.tile([P, 1], f32)
        nc.vector.memset(eps_t[:], EPS)

        # ---- persistent activations ----
        x = xp.tile([P, 2, D], f32)            # fp32 residual, token-major
        nc.sync.dma_start(x[:], dp["x0"][:])
        hT = ap_.tile([P, KT, NTOK], f16)      # LN'd x, feature-major
        qT = ap_.tile([P, KT, NTOK], f16)
        kT_own = ap_.tile([P, KT, NTOK], f8)
        kT_all = ap_.tile([P, KT, NG, NTOK], f8)
        v_own = ap_.tile([P, 2, H, DH + 1], f16)
        v_ext = ap_.tile([P, 2 * NG, H, DH + 1], f16)
        oT = ap_.tile([P, KT, NTOK], f16)
        h1g = ap_.tile([P, FC1 * NTOK], f16)
        h1gT = h1g[:].rearrange("p (fc t) -> p fc t", fc=FC1)
        stage = ap_.tile([P, 2, D], f16)       # h16 / o16 staging

        # ================= layers =================
        for l in range(L):
            # ---- weight loads (scalar queue; single-buffered => prefetch)
            wq_t = wp.tile([P, KT, D], f16, tag="wq")
            nc.scalar.dma_start(wq_t[:], dp["wq"][l])
            wk_t = wp.tile([P, KT, D], f16, tag="wk")
            nc.scalar.dma_start(wk_t[:], dp["wk"][l])
            wv_t = wp.tile([P, KT, D], f16, tag="wv")
            nc.scalar.dma_start(wv_t[:], dp["wv"][l])
            wo_t = wp.tile([P, KT, D], f16, tag="wo")
            nc.scalar.dma_start(wo_t[:], dp["wo"][l])
            w1_t = wp.tile([P, KT, DFF], f16, tag="w1")
            nc.scalar.dma_start(w1_t[:], dp["w1"][l])
            w2_t = wp.tile([P, FC1, D], f16, tag="w2")
            nc.scalar.dma_start(w2_t[:], dp["w2"][l])
            bq_t = wp.tile([P, KT], f32, tag="bq")
            nc.scalar.dma_start(bq_t[:], dp["bq"][l])
            bk_t = wp.tile([P, KT], f32, tag="bk")
            nc.scalar.dma_start(bk_t[:], dp["bk"][l])
            bv_t = wp.tile([P, D], f16, tag="bv")
            nc.scalar.dma_start(bv_t[:], dp["bv"][l, None, :].to_broadcast((P, D)))
            b1_t = wp.tile([P, FC1], f32, tag="b1")
            nc.scalar.dma_start(b1_t[:], dp["b1"][l])

            # ---- LN1 + transpose ----
            _layernorm(nc, sp, eps_t, x, stage)
            h_dram = dmp.tile([NTOK, D], f16, tag="h_dram")
            nc.sync.dma_start(h_dram[:].rearrange("(tb p) d -> p tb d", p=P), stage[:])
            for kt in range(KT):
                nc.sync.dma_start_transpose(hT[:, kt, :], h_dram[:, kt * P:(kt + 1) * P])

            # ---- k projection (feature-major) ----
            for fc in range(KT):
                pk = psA.tile([P, 512], f32, tag="psA")
                for kt in range(KT):
                    nc.tensor.matmul(
                        pk[:, 0:NTOK], wk_t[:, kt, fc * P:(fc + 1) * P], hT[:, kt, :],
                        start=(kt == 0), stop=(kt == KT - 1))
                nc.vector.tensor_scalar_add(
                    kT_own[:, fc, :], pk[:, 0:NTOK], bk_t[:, fc:fc + 1])

            # ---- K AllGather (fp8), issued before the v projection ----
            k_in = dmp.tile([KSZ], f8, tag="k_in")
            nc.sync.dma_start(
                k_in[:].rearrange("(fc p t) -> p fc t", p=P, t=NTOK), kT_own[:])
            k_out = dmp.tile([NG, KSZ], f8, tag="k_out")
            if not NO_COMM:
                nc.gpsimd.collective_compute(
                    "AllGather", ALU.bypass, replica_groups=GROUPS,
                    ins=[k_in[:]], outs=[k_out[:]],
                )
            for s in range(NG):
                nc.scalar.dma_start(
                    kT_all[:, :, s, :],
                    k_out[s, :].rearrange("(fc p t) -> p fc t", p=P, t=NTOK))

            # ---- v projection (token-major) into v_own staging ----
            nc.vector.memset(v_own[:, :, :, DH], 1.0)
            for tb in range(2):
                for c in range(2):
                    pv = psA.tile([P, 512], f32, tag="psA")
                    for kt in range(KT):
                        nc.tensor.matmul(
                            pv[:], hT[:, kt, tb * P:(tb + 1) * P],
                            wv_t[:, kt, c * 512:(c + 1) * 512],
                            start=(kt == 0), stop=(kt == KT - 1))
                    nc.vector.tensor_add(
                        v_own[:, tb, 8 * c:8 * (c + 1), 0:DH],
                        pv[:].rearrange("p (h d) -> p h d", h=8),
                        bv_t[:, c * 512:(c + 1) * 512].rearrange("p (h d) -> p h d", h=8))

            # ---- V AllGather (fp16) ----
            v_in = dmp.tile([VSZ], f16, tag="v_in")
            nc.sync.dma_start(
                v_in[:].rearrange("(s p h e) -> p s h e", p=P, s=2, h=H),
                v_own[:])
            v_out = dmp.tile([NG, VSZ], f16, tag="v_out")
            if not NO_COMM:
                nc.gpsimd.collective_compute(
                    "AllGather", ALU.bypass, replica_groups=GROUPS,
                    ins=[v_in[:]], outs=[v_out[:]],
                )

            # ---- q projection (overlaps the AllGathers) ----
            for fc in range(KT):
                pq = psA.tile([P, 512], f32, tag="psA")
                for kt in range(KT):
                    nc.tensor.matmul(
                        pq[:, 0:NTOK], wq_t[:, kt, fc * P:(fc + 1) * P], hT[:, kt, :],
                        start=(kt == 0), stop=(kt == KT - 1))
                nc.vector.tensor_scalar_add(
                    qT[:, fc, :], pq[:, 0:NTOK], bq_t[:, fc:fc + 1])

            # ---- read back v shards (sync queue: nothing early blocks on it)
            for s in range(NG):
                nc.sync.dma_start(
                    v_ext[:, 2 * s:2 * s + 2, :, :],
                    v_out[s, :].rearrange(
                        "(s2 p h e) -> p s2 h e", p=P, s2=2, h=H))

            # ---- attention ----
            # q-tile 0 attends slots {2s} (diag blocks); q-tile 1 attends
            # slots {2s} (full) + {2s+1} (diag).  slot = 2s + j.
            for fc in range(KT):           # head pair (2fc, 2fc+1)
                E16 = ep.tile([P, 2 * NG, 2, NTOK], f16, tag="E16")
                for s in range(NG):
                    psc = psSC.tile([P, 2, 512], f32, tag="psSC")
                    for h2 in range(2):
                        nc.tensor.matmul(
                            psc[:, h2, 0:NTOK],
                            kT_all[64 * h2:64 * h2 + 64, fc, s, 0:P],
                            qT[64 * h2:64 * h2 + 64, fc, :],
                            start=True, stop=True)
                    nc.scalar.activation(
                        E16[:, 2 * s, :, :], psc[:, :, 0:NTOK],
                        AF.Exp, scale=float(1.0 / np.sqrt(DH)))
                    psc2 = psSC.tile([P, 2, 512], f32, tag="psSC")
                    for h2 in range(2):
                        nc.tensor.matmul(
                            psc2[:, h2, 0:P],
                            kT_all[64 * h2:64 * h2 + 64, fc, s, P:NTOK],
                            qT[64 * h2:64 * h2 + 64, fc, P:NTOK],
                            start=True, stop=True)
                    nc.scalar.activation(
                        E16[:, 2 * s + 1, :, P:NTOK], psc2[:, :, 0:P],
                        AF.Exp, scale=float(1.0 / np.sqrt(DH)))
                    for h2 in range(2):
                        nc.vector.tensor_mul(
                            E16[:, 2 * s, h2, 0:P],
                            E16[:, 2 * s, h2, 0:P], maskd[:, s, :])
                        nc.vector.tensor_mul(
                            E16[:, 2 * s + 1, h2, P:NTOK],
                            E16[:, 2 * s + 1, h2, P:NTOK], maskd[:, s, :])
                for h2 in range(2):
                    h = 2 * fc + h2
                    for tb in range(2):
                        slots = ([2 * s for s in range(NG)] if tb == 0 else
                                 list(range(2 * NG)))
                        po = psAV.tile([P, DH + 1], f32, tag="psAV")
                        for idx, slot in enumerate(slots):
                            nc.tensor.matmul(
                                po[:], E16[:, slot, h2, tb * P:(tb + 1) * P],
                                v_ext[:, slot, h, :],
                                start=(idx == 0), stop=(idx == len(slots) - 1))
                        rn = sp.tile([P, 1], f32, tag="rn")
                        nc.vector.reciprocal(rn[:], po[:, DH:DH + 1])
                        nc.vector.tensor_scalar_mul(
                            stage[:, tb, h * DH:(h + 1) * DH], po[:, 0:DH], rn[:])

            # ---- o transpose + Wout + residual ----
            o_dram = dmp.tile([NTOK, D], f16, tag="h_dram")
            nc.sync.dma_start(o_dram[:].rearrange("(tb p) d -> p tb d", p=P), stage[:])
            for oc in range(KT):
                nc.scalar.dma_start_transpose(oT[:, oc, :], o_dram[:, oc * P:(oc + 1) * P])
            for tb in range(2):
                for c in range(2):
                    pw = psA.tile([P, 512], f32, tag="psA")
                    for oc in range(KT):
                        nc.tensor.matmul(
                            pw[:], oT[:, oc, tb * P:(tb + 1) * P],
                            wo_t[:, oc, c * 512:(c + 1) * 512],
                            start=(oc == 0), stop=(oc == KT - 1))
                    nc.vector.tensor_add(
                        x[:, tb, c * 512:(c + 1) * 512],
                        x[:, tb, c * 512:(c + 1) * 512], pw[:])

            # ---- LN2 + transpose ----
            _layernorm(nc, sp, eps_t, x, stage)
            h2_dram = dmp.tile([NTOK, D], f16, tag="h_dram")
            nc.sync.dma_start(h2_dram[:].rearrange("(tb p) d -> p tb d", p=P), stage[:])
            for kt in range(KT):
                nc.sync.dma_start_transpose(hT[:, kt, :], h2_dram[:, kt * P:(kt + 1) * P])

            # ---- FFN ----
            for fc in range(FC1):
                pf = psA.tile([P, 512], f32, tag="psA")
                for kt in range(KT):
                    nc.tensor.matmul(
                        pf[:, 0:NTOK], w1_t[:, kt, fc * P:(fc + 1) * P], hT[:, kt, :],
                        start=(kt == 0), stop=(kt == KT - 1))
                nc.scalar.activation(
                    h1gT[:, fc, :], pf[:, 0:NTOK],
                    AF.Sigmoid if SIM_GELU_SUBST else AF.Gelu,
                    bias=b1_t[:, fc:fc + 1])
            for tb in range(2):
                for c in range(2):
                    pw = psA.tile([P, 512], f32, tag="psA")
                    for fc in range(FC1):
                        nc.tensor.matmul(
                            pw[:], h1gT[:, fc, tb * P:(tb + 1) * P],
                            w2_t[:, fc, c * 512:(c + 1) * 512],
                            start=(fc == 0), stop=(fc == FC1 - 1))
                    nc.vector.tensor_add(
                        x[:, tb, c * 512:(c + 1) * 512],
                        x[:, tb, c * 512:(c + 1) * 512], pw[:])

        # ---- final layernorm + output ----
        mv = sp.tile([P, 2, 2], f32, tag="ln_mv")
        for tb in range(2):
            st = sp.tile([P, 2, 6], f32, tag="ln_st")
            for g in range(2):
                nc.vector.bn_stats(st[:, g, :], x[:, tb, g * 512:(g + 1) * 512])
            nc.vector.bn_aggr(mv[:, tb, :], st[:])
        std = sp.tile([P, 2], f32, tag="ln_std")
        nc.scalar.activation(std[:], mv[:, :, 1], AF.Sqrt, bias=eps_t[:])
        rstd = sp.tile([P, 2], f32, tag="ln_rstd")
        nc.vector.reciprocal(rstd[:], std[:])
        for tb in range(2):
            nc.vector.tensor_scalar(
                stage[:, tb, :], x[:, tb, :], mv[:, tb, 0:1], rstd[:, tb:tb + 1],
                ALU.subtract, ALU.mult)
            nc.vector.tensor_mul(stage[:, tb, :], stage[:, tb, :], lnf_t[:, 0, :])
            nc.vector.tensor_add(stage[:, tb, :], stage[:, tb, :], lnf_t[:, 1, :])
            nc.sync.dma_start(dp["out"][tb * P:(tb + 1) * P, :], stage[:, tb, :])


# ======================= host side =======================

def _prep_inputs(input_ids, token_emb, pos_emb, ln1_s, ln1_b, Wqkv, Wout,
                 ln2_s, ln2_b, W1, W2, lnf_s, lnf_b):
    ids = np.asarray(input_ids)
    emb = np.asarray(token_emb, np.float32)
    pos = np.asarray(pos_emb, np.float32)
    Wqkv64 = np.asarray(Wqkv, np.float64)
    W164 = np.asarray(W1, np.float64)
    Wqkv_f = Wqkv64 * np.asarray(ln1_s, np.float64)[:, :, None]
    bqkv_f = np.einsum("ld,ldn->ln", np.asarray(ln1_b, np.float64), Wqkv64)
    w1 = (W164 * np.asarray(ln2_s, np.float64)[:, :, None]).astype(np.float16)
    b1 = np.einsum("ld,ldn->ln", np.asarray(ln2_b, np.float64), W164).astype(np.float32)
    def colmajor(w, nchunk):
        # [L, nchunk*P, N] -> [L, P, nchunk, N] so SBUF loads are contiguous
        Lw, _, N = w.shape
        return np.ascontiguousarray(
            w.reshape(Lw, nchunk, P, N).transpose(0, 2, 1, 3))

    def bias_prep(b, nchunk):
        Lb = b.shape[0]
        return np.ascontiguousarray(b.reshape(Lb, nchunk, P).transpose(0, 2, 1))

    shared = dict(
        wq=colmajor(Wqkv_f[:, :, 0:D].astype(np.float16), KT),
        wk=colmajor(Wqkv_f[:, :, D:2 * D].astype(np.float16), KT),
        wv=colmajor(Wqkv_f[:, :, 2 * D:3 * D].astype(np.float16), KT),
        wo=colmajor(np.asarray(Wout, np.float16), KT),
        w1=colmajor(w1, KT),
        w2=colmajor(np.asarray(W2, np.float16), FC1),
        bq=bias_prep(bqkv_f[:, 0:D].astype(np.float32), KT),
        bk=bias_prep(bqkv_f[:, D:2 * D].astype(np.float32), KT),
        bv=bqkv_f[:, 2 * D:3 * D].astype(np.float16),
        b1=bias_prep(b1, FC1),
        lnf=np.stack([np.asarray(lnf_s), np.asarray(lnf_b)]).astype(np.float16),
    )

    x_full = emb[ids] + pos[None, :T]          # (B, T, D) f32
    bidx = np.arange(P)
    in_maps = []
    for core in range(NCORES):
        g, r = divmod(core, NG)
        x0 = np.ascontiguousarray(
            x_full[g, r::NG].reshape(2, P, D).transpose(1, 0, 2)).astype(np.float32)
        # diag-block mask: key row b (shard s), query col a (shard r):
        # valid iff 4b+s <= 4a+r  <=>  b < a or (b == a and s <= r)
        mask = np.empty((NG, P, P), np.float16)
        for s in range(NG):
            m = (bidx[:, None] < bidx[None, :]) | (
                (bidx[:, None] == bidx[None, :]) & (s <= r))
            mask[s] = m.astype(np.float16)
        mask = np.ascontiguousarray(mask.transpose(1, 0, 2))  # [P, NG, P]
        in_maps.append(dict(shared, x0=x0, mask=mask))
    return in_maps


# ---------- compile-once / run-many PJRT runner (vendored) ----------

class SpmdRunner:
    def __init__(self, nc, n_cores=8):
        import jax
        from jax.sharding import Mesh, PartitionSpec
        from jax.experimental.shard_map import shard_map
        from concourse.bass2jax import (
            _bass_exec_p, install_neuronx_cc_hook, partition_id_tensor)
        self.jax = jax
        self.PartitionSpec = PartitionSpec
        install_neuronx_cc_hook()
        if not nc.is_finalized():
            nc.finalize()
        self.n_cores = n_cores
        partition_name = (
            nc.partition_id_tensor.name if nc.partition_id_tensor else None)
        in_names, out_names, out_avals, zero_outs = [], [], [], []
        for alloc in nc.m.functions[0].allocations:
            if not isinstance(alloc, mybir.MemoryLocationSet):
                continue
            name = alloc.memorylocations[0].name
            if alloc.kind == "ExternalInput":
                if name != partition_name:
                    in_names.append(name)
            elif alloc.kind == "ExternalOutput":
                out_names.append(name)
                shape = tuple(alloc.tensor_shape)
                dtype = mybir.dt.np(alloc.dtype)
                out_avals.append(jax.core.ShapedArray(shape, dtype))
                zero_outs.append(np.zeros(shape, dtype))
        self.in_names, self.out_names = in_names, out_names
        self.out_avals, self.zero_outs = out_avals, zero_outs
        n_params, n_outs = len(in_names), len(out_avals)
        self.n_params = n_params
        all_in = in_names + out_names + (
            [partition_name] if partition_name else [])
        donate = tuple(range(n_params, n_params + n_outs))

        def _b(*args):
            ops = list(args)
            if partition_name:
                ops.append(partition_id_tensor())
            return tuple(_bass_exec_p.bind(
                *ops, out_avals=tuple(out_avals), in_names=tuple(all_in),
                out_names=tuple(out_names), lowering_input_output_aliases=(),
                sim_require_finite=True, sim_require_nnan=True, nc=nc))

        devices = jax.devices()[:n_cores]
        self.mesh = Mesh(np.asarray(devices), ("core",))
        specs = (PartitionSpec("core"),)
        self.sharded = jax.jit(
            shard_map(_b, mesh=self.mesh,
                      in_specs=specs * (n_params + n_outs),
                      out_specs=specs * len(out_names), check_rep=False),
            donate_argnums=donate, keep_unused=True)
        self._dev_inputs = None

    def _zeros(self):
        return [np.zeros((self.n_cores * z.shape[0], *z.shape[1:]), z.dtype)
                for z in self.zero_outs]

    def stage_inputs(self, in_maps):
        jax, PS = self.jax, self.PartitionSpec
        per_core = [[np.asarray(m[n]) for n in self.in_names] for m in in_maps]
        concat = [np.concatenate([per_core[c][i] for c in range(self.n_cores)],
                                 axis=0) for i in range(self.n_params)]
        sh = jax.sharding.NamedSharding(self.mesh, PS("core"))
        self._dev_inputs = [jax.device_put(a, sh) for a in concat]
        for a in self._dev_inputs:
            a.block_until_ready()

    def run(self, in_maps=None):
        if in_maps is not None:
            self.stage_inputs(in_maps)
        outs = self.sharded(*self._dev_inputs, *self._zeros())
        out_np = [np.asarray(a) for a in outs]
        return [{n: out_np[i].reshape(self.n_cores, *self.out_avals[i].shape)[c]
                 for i, n in enumerate(self.out_names)}
                for c in range(self.n_cores)]

    def time_exec(self, iters=8, warmup=2):
        jax, PS = self.jax, self.PartitionSpec
        sh = jax.sharding.NamedSharding(self.mesh, PS("core"))
        zsets = [[jax.device_put(z, sh) for z in self._zeros()]
                 for _ in range(warmup + iters)]
        for zs in zsets:
            for z in zs:
                z.block_until_ready()
        outs = []
        for i in range(warmup):
            outs.append(self.sharded(*self._dev_inputs, *zsets[i]))
        for o in outs[-1]:
            o.block_until_ready()
        t0 = time.perf_counter()
        outs = []
        for i in range(iters):
            outs.append(self.sharded(*self._dev_inputs, *zsets[warmup + i]))
        for o in outs[-1]:
            o.block_until_ready()
        return (time.perf_counter() - t0) / iters


_RUNNER = None


def get_runner():
    global _RUNNER
    if _RUNNER is None:
        _RUNNER = SpmdRunner(build_nc(), NCORES)
    return _RUNNER


def kernel(**inputs) -> np.ndarray:
    in_maps = _prep_inputs(**{k: np.asarray(v) for k, v in inputs.items()})
    res = get_runner().run(in_maps)
    out = np.empty((B, T, D), np.float32)
    for core in range(NCORES):
        g, r = divmod(core, NG)
        out[g, r::NG] = res[core]["out"].astype(np.float32)
    return out
